# revision 58
# baseline (speedup 1.0000x reference)
"""DiT block kernel for TRN2, 8 NeuronCores.

Sharding: DP=4 over batch x TP=2 over heads (Megatron style).
Core c: batch b=c//2, half hf=c%2 (8 of 16 heads, 2048/4096 MLP cols, 512/1024
rows of the row-parallel weights).

Device layout is feature-major: activations [feature(partitions), token(free)], so
every matmul chains without transposes: outT = W.T @ actT with lhsT=W natural layout.
v^T (lhsT of attn@v) is produced by swapping matmul operands. Softmax runs without
max-subtraction (scores are O(1)); denominators come from an augmented ones-column in
v^T (row 64 of the [65, tok] attention output).

Dtypes (cost model: fp8e4+DoubleRow matmul = 0.5 cyc/row over TWO k-tiles = 4x bf16
MAC rate; DVE 2x for all-16-bit ops; DMA is a serialized shared device):
 - fp8e4 + DoubleRow: qkv, v, self-attn scores (32-partition split-head layout),
   attn@v, q2/kc/vc, fc1 (3 DR terms: h8@W8 + (h3-h8)@W8 + (h3/64)@(64*(W-W8)) --
   hi/lo splits on BOTH operands keep fc1 at bf16-level accuracy).
 - bf16: W_ada, W_ctx/te, proj, out, fc2, AllReduce buffers (branch-only), hidden, at.
 - fp32: trunk x (residuals accumulate locally; AllReduce carries branch partials
   only), PSUM, LN stats, mod.

Self-attn q/k use a permuted column layout: chunk c=2g+b holds dims [32b:32b+32)
of heads 4g..4g+3, so scores contract HD=64 as a DoubleRow pair of 32-partition
tiles at base partitions {0,32,64,96}.


Comms: 3 pair AllReduces (adaLN mod fp32, self-attn proj+residual bf16, cross-attn
out+residual bf16); the fc2 reduce is folded into the output (host adds partials).
"""
import sys
import numpy as np

sys.path.insert(0, "/opt/trn_rl_repo")

import ml_dtypes
import concourse.bass as bass
import concourse.mybir as mybir
import concourse.tile as tile
from concourse import bacc
from concourse.bass_utils import run_bass_kernel_spmd

FP32 = mybir.dt.float32
FP32R = mybir.dt.float32r
BF16 = mybir.dt.bfloat16
FP8 = mybir.dt.float8e4
AF = mybir.ActivationFunctionType
ALU = mybir.AluOpType
DRM = mybir.MatmulPerfMode.DoubleRow
I32 = mybir.dt.int32
EXP_A = 12102203.161561485   # 2^23 / ln2 (Schraudolph fast-exp)
EXP_B = 1064985216.0         # 127*2^23 - 368000, exactly representable in fp32

B, N, D, H, TD, TL = 4, 1024, 1024, 16, 768, 77
HD = 64
EPS = 1e-6
HL = 8
DL = 512
FFL = 2048
T = 1024
TLP = 80
NCH = D // 128
REPLICA_GROUPS = [[0, 1], [2, 3], [4, 5], [6, 7]]


def _declare(nc):
    d = {}

    def inp(name, shape, dt):
        d[name] = nc.dram_tensor(name, list(shape), dt, kind="ExternalInput").ap()

    inp("xT", (D, T), BF16)
    inp("cT", (128, 4), FP32)
    inp("teT", (TD, TLP), BF16)
    inp("w_ada", (12, 128, 4, 512), BF16)
    inp("b_adaT", (128, 6, 8), FP32)
    inp("w_qkv", (6, 128, 8, 256), FP8)
    inp("b_qkT", (128, 8), FP32)
    inp("b_v", (1, DL), FP32R)
    inp("w_proj", (2, 128, 4, 512), BF16)
    inp("b_projT", (128, 8), FP32)
    inp("w_ctx", (4, 128, 6, 256), BF16)
    inp("b_ctxT", (128, 8), FP32)
    inp("w_q", (2, 128, 8, 256), FP8)
    inp("w_k", (2, 128, 8, 256), FP8)
    inp("w_v", (2, 128, 8, 256), FP8)
    inp("w_out", (2, 128, 4, 512), BF16)
    inp("b_outT", (128, 8), FP32)
    inp("w_fc1h", (8, 128, 8, 256), FP8)
    inp("w_fc1l", (8, 128, 8, 256), FP8)
    inp("b_fc1T", (128, 16), FP32)
    inp("w_fc2", (4, 128, 16, 256), BF16)
    inp("b_fc2T", (128, 8), FP32)
    inp("ones_r", (128, 128), FP32R)
    inp("ones_b", (128, 8), BF16)
    inp("maskT", (128, 1), FP32)
    d["out_xT"] = nc.dram_tensor("out_xT", [D, T], BF16, kind="ExternalOutput").ap()
    return d


def _emit(tc, io, pools, nocc=False):
    nc = tc.nc
    sb = pools["sb"]
    xp, hp, qkp, vtp, atp, pp = (pools[k] for k in ("xp", "hp", "qkp", "vtp", "atp", "pp"))
    wst, wst2, hidp, xsqp, scr, rcpp = (pools[k] for k in
                                        ("wst", "wst2", "hidp", "xsqp", "scr", "rcpp"))
    vecp = pools["vecp"]
    ps_mm, ps_aux = pools["ps_mm"], pools["ps_aux"]
    dram = pools["dram"]

    ones = sb.tile([128, 128], FP32R, tag="ones")
    nc.sync.dma_start(out=ones, in_=io["ones_r"])
    ones_b = sb.tile([128, 8], BF16, tag="ones_b")
    nc.sync.dma_start(out=ones_b, in_=io["ones_b"])
    mask = sb.tile([128, 1], FP32, tag="mask")
    nc.sync.dma_start(out=mask, in_=io["maskT"])

    def load_wblock(src_ap, dt, bi, pool):
        _, _, kc, mblk = src_ap.shape
        wt = pool.tile([128, kc, mblk], dt, tag="w")
        nc.sync.dma_start(out=wt, in_=src_ap[bi])
        return wt

    # ---------------- Stage 0: adaLN ----------------
    cT = sb.tile([128, 4], FP32, tag="cT")
    nc.sync.dma_start(out=cT, in_=io["cT"])
    cs = sb.tile([128, 4], BF16, tag="cs")
    nc.scalar.activation(cs, cT, AF.Silu)

    ar_mod_in = dram.tile([1, 6 * D], BF16, tag="armod_i")
    ar_mod_out = dram.tile([1, 6 * D], BF16, tag="armod_o")

    b_adaT = sb.tile([128, 6, 8], FP32, tag="b_adaT")
    nc.sync.dma_start(out=b_adaT, in_=io["b_adaT"])
    modT = sb.tile([128, 6, 8], FP32, tag="modT")
    ksc = sb.tile([128, 3, 8], FP32, tag="ksc")
    mod_view = ar_mod_out.rearrange("o (g j p) -> (o p) g j", p=128, g=6)

    def ada_blocks(mbs):
        for mb in mbs:   # w_ada rhs-blocks [512, 512]
            wt = load_wblock(io["w_ada"], BF16, mb, wst)
            pm = (ps_aux if mb % 2 == 0 else ps_mm).tile([1, 512], FP32,
                                                         tag="aux" if mb % 2 == 0 else "mm")
            for k in range(4):
                nc.tensor.matmul(pm, cs[:, k:k + 1], wt[:, k, :],
                                 start=(k == 0), stop=(k == 3))
            mp = vecp.tile([1, 512], BF16, tag="tmpv", name=f"modp_{mb}")
            nc.vector.tensor_copy(mp, pm)
            nc.sync.dma_start(out=ar_mod_in[:, 512 * mb:512 * (mb + 1)], in_=mp)

    def mod_reduce(c0, c1, g0, g1):
        if nocc:
            nc.sync.dma_start(out=ar_mod_out[:, c0:c1], in_=ar_mod_in[:, c0:c1])
        else:
            nc.gpsimd.collective_compute(
                "AllReduce", ALU.add, replica_groups=REPLICA_GROUPS,
                ins=[ar_mod_in[:, c0:c1].opt()], outs=[ar_mod_out[:, c0:c1].opt()])
        mtmp = sb.tile([128, 6, 8], BF16, tag="mtmp", name=f"mtmp{g0}")
        nc.sync.dma_start(out=mtmp[:, g0:g1, :], in_=mod_view[:, g0:g1, :])
        nc.vector.tensor_tensor(modT[:, g0:g1, :], mtmp[:, g0:g1, :],
                                b_adaT[:, g0:g1, :], op=ALU.add)
        for i in range(g0 // 2, g1 // 2):
            nc.vector.tensor_scalar(ksc[:, i, :], modT[:, 2 * i + 1, :], 1.0, None,
                                    op0=ALU.add)

    x1 = []
    for j in range(NCH):
        xt = xp.tile([128, T], BF16, tag="x")
        nc.sync.dma_start(out=xt, in_=io["xT"][128 * j:128 * (j + 1), :])
        x1.append(xt)
    ada_blocks(range(4))
    mod_reduce(0, 2048, 0, 2)
    eps_t = sb.tile([1, 1], FP32, tag="eps")
    nc.vector.memset(eps_t, EPS)

    # ---------------- LN + modulate -> fp8 h pairs ----------------
    def layer_norm_mod(x_tiles, g_sh, g_sc, lo=False):
        bf_x = g_sh == 0
        lones = ones_b if bf_x else ones
        sum_ps = ps_aux.tile([1, T], FP32, tag="aux")
        sq_ps = ps_aux.tile([1, T], FP32, tag="aux")
        for j in range(NCH):
            xsq = xsqp.tile([128, T], BF16 if bf_x else FP32R, tag="xsq")
            if bf_x:
                nc.vector.tensor_tensor(xsq, x_tiles[j], x_tiles[j], op=ALU.mult)
            else:
                nc.scalar.activation(xsq, x_tiles[j], AF.Square)
            for nb in range(2):
                s = slice(512 * nb, 512 * (nb + 1))
                nc.tensor.matmul(sum_ps[:, s], lones[:, 0:1], x_tiles[j][:, s],
                                 start=(j == 0), stop=(j == NCH - 1), skip_group_check=True)
                nc.tensor.matmul(sq_ps[:, s], lones[:, 1:2], xsq[:, s],
                                 start=(j == 0), stop=(j == NCH - 1), skip_group_check=True)
        mu = vecp.tile([1, T], FP32R, tag="mu")
        nc.scalar.activation(mu, sum_ps, AF.Copy, scale=1.0 / D)
        musq = vecp.tile([1, T], FP32, tag="tmpv")
        nc.vector.tensor_tensor(musq, mu, mu, op=ALU.mult)
        var = vecp.tile([1, T], FP32, tag="tmpv2")
        nc.vector.scalar_tensor_tensor(var, sq_ps, 1.0 / D, musq,
                                       op0=ALU.mult, op1=ALU.subtract)
        sig = vecp.tile([1, T], FP32, tag="tmpv")
        nc.scalar.activation(sig, var, AF.Sqrt, bias=eps_t)
        rsig = vecp.tile([1, T], FP32R, tag="rsig")
        with nc.allow_low_precision(reason="fp32r rsig feeds fp32r broadcast matmul"):
            nc.vector.reciprocal(rsig, sig)
        rs_b = ps_aux.tile([128, T], FP32, tag="aux")
        mrs_b = ps_aux.tile([128, T], FP32, tag="aux")
        for nb in range(2):
            s = slice(512 * nb, 512 * (nb + 1))
            nc.tensor.matmul(rs_b[:, s], ones[0:1, :], rsig[:, s], start=True, stop=True)
        nmrs = vecp.tile([1, T], FP32R, tag="tmpv", name="nmrs")
        with nc.allow_low_precision(reason="fp32r nmrs feeds fp32r broadcast matmul"):
            nc.vector.scalar_tensor_tensor(nmrs, mu, -1.0, rsig,
                                           op0=ALU.mult, op1=ALU.mult)
        rs_sb = xsqp.tile([128, T], BF16, tag="xsq", name=f"rs_sb{g_sh}")
        nc.scalar.copy(rs_sb, rs_b)
        for nb in range(2):
            s = slice(512 * nb, 512 * (nb + 1))
            nc.tensor.matmul(mrs_b[:, s], ones[0:1, :], nmrs[:, s], start=True, stop=True)
        mrs_sb = xsqp.tile([128, T], BF16, tag="xsq", name=f"mrs_sb{g_sh}")
        nc.scalar.copy(mrs_sb, mrs_b)
        h_tiles = [hp.tile([128, 2, T], FP8, tag="h", name=f"h{g_sh}_{p}")
                   for p in range(4)]
        hd_tiles = hl_tiles = None
        if lo:
            hd_tiles = [hp.tile([128, 2, T], FP8, tag="h", name=f"hd_{p}")
                        for p in range(4)]
            hl_tiles = [hp.tile([128, 2, T], FP8, tag="h", name=f"hl_{p}")
                        for p in range(4)]
            kscd = sb.tile([128, 8], FP32, tag="kscd")
            nc.vector.tensor_scalar(kscd, ksc[:, g_sc, :], 1.0 / 64, None, op0=ALU.mult)
            shd = sb.tile([128, 8], FP32, tag="shd")
            nc.vector.tensor_scalar(shd, modT[:, g_sh, :], 1.0 / 64, None, op0=ALU.mult)

        for j in range(NCH):
            u = scr.tile([128, T], BF16, tag="t1")
            nc.vector.tensor_tensor(u, x_tiles[j], rs_sb, op=ALU.mult)
            nc.vector.tensor_tensor(u, u, mrs_sb, op=ALU.add)
            h8 = h_tiles[j // 2][:, j % 2, :]
            nc.scalar.activation(h8, u, AF.Identity,
                                 scale=ksc[:, g_sc, j:j + 1],
                                 bias=modT[:, g_sh, j:j + 1])
            if lo:
                nc.scalar.activation(hd_tiles[j // 2][:, j % 2, :], u, AF.Identity,
                                     scale=kscd[:, j:j + 1], bias=shd[:, j:j + 1])
                h3s = scr.tile([128, T], FP32, tag="h3s")
                nc.scalar.activation(h3s, u, AF.Identity,
                                     scale=ksc[:, g_sc, j:j + 1],
                                     bias=modT[:, g_sh, j:j + 1])
                nc.vector.scalar_tensor_tensor(hl_tiles[j // 2][:, j % 2, :],
                                               h8, -1.0, h3s,
                                               op0=ALU.mult, op1=ALU.add)
        if lo:
            return h_tiles, hd_tiles, hl_tiles
        return h_tiles

    # ---------------- Stage 1: LN1 ----------------
    h1 = layer_norm_mod(x1, g_sh=0, g_sc=0)

    # ---------------- Stage 2: qkv + vT ----------------
    b_qkT = sb.tile([128, 8], FP32, tag="b_qkT")
    nc.sync.dma_start(out=b_qkT, in_=io["b_qkT"])
    bvr = sb.tile([1, DL], FP32R, tag="bv")
    nc.sync.dma_start(out=bvr, in_=io["b_v"])
    # q,k: split-head layout [128=4heads x 32dims, 2 halves, T], fp8 DoubleRow
    qg = [qkp.tile([128, 2, T], FP8, tag="qk", name=f"qg{g}") for g in range(2)]
    kg = [qkp.tile([128, 2, T], FP8, tag="qk", name=f"kg{g}") for g in range(2)]
    for mb in range(4):
        wt = load_wblock(io["w_qkv"], FP8, mb, wst)
        for mm in range(2):
            c = 2 * mb + mm
            pm = ps_mm.tile([128, T], FP32, tag="mm")
            for nb in range(2):
                s = slice(512 * nb, 512 * (nb + 1))
                for k2 in range(4):
                    nc.tensor.matmul(pm[:, s], wt[:, 2 * k2:2 * k2 + 2, 128 * mm:128 * (mm + 1)],
                                     h1[k2][:, :, s], start=(k2 == 0), stop=(k2 == 3),
                                     perf_mode=DRM)
            tgt = qg if c < 4 else kg
            g, b = (c % 4) // 2, c % 2
            nc.scalar.activation(tgt[g][:, b, :], pm, AF.Identity,
                                 bias=b_qkT[:, c:c + 1])
    # vT: token-major pairs [128, 2, HL, HD+1] with ones-augmented head columns
    wv_blk = [load_wblock(io["w_qkv"], FP8, 4 + g, wst) for g in range(2)]
    vT = []
    for mp_ in range(4):
        vt = vtp.tile([128, 2, HL, HD + 2], FP8, tag="vt")
        for half in range(2):
            m = 2 * mp_ + half
            pm = ps_aux.tile([128, DL], FP32, tag="aux")
            for g in range(2):
                for k2 in range(4):
                    nc.tensor.matmul(pm[:, 256 * g:256 * (g + 1)],
                                     h1[k2][:, :, 128 * m:128 * (m + 1)],
                                     wv_blk[g][:, 2 * k2:2 * k2 + 2, :],
                                     start=(k2 == 0), stop=False, perf_mode=DRM)
                nc.tensor.matmul(pm[:, 256 * g:256 * (g + 1)], ones[0:1, :],
                                 bvr[:, 256 * g:256 * (g + 1)], start=False, stop=(g == 1),
                                 skip_group_check=True)
            nc.scalar.copy(vt[:, half, :, 0:HD],
                           pm.rearrange("p (a b) -> p a b", a=HL))
            nc.vector.memset(vt[:, half, :, HD:HD + 1], 1.0)
        vT.append(vt)

    # ---------------- self-attention (fp8, DR scores + DR av) ----------------
    def normalize_head(at_tiles, h, po, act_copy=False):
        ti, off = h // 2, 64 * (h % 2)
        rcp = rcpp.tile([1, T], FP32R, tag="rcp")
        with nc.allow_low_precision(reason="fp32r rcp feeds fp32r broadcast matmul"):
            nc.vector.reciprocal(rcp, po[64:65, :])
        pb = ps_mm.tile([64, T], FP32, tag="mm")
        for nb in range(2):
            s = slice(512 * nb, 512 * (nb + 1))
            nc.tensor.matmul(pb[:, s], ones[0:1, 0:64], rcp[:, s], start=True, stop=True)
        rc_sb = rcpp.tile([64, T], BF16, tag="rc_sb")
        if act_copy:
            nc.scalar.copy(rc_sb, pb)
        else:
            nc.vector.tensor_copy(rc_sb, pb)
        nc.vector.tensor_tensor(at_tiles[ti][off:off + 64, :], po[0:64, :], rc_sb,
                                op=ALU.mult)

    def attention_self(qg, kg, vT, filler=None):
        at_tiles = [atp.tile([128, T], BF16, tag="at", name=f"at_{i}") for i in range(4)]
        pend = None
        for h in range(HL):
            if filler is not None:
                filler(h)
            g, i = h // 4, h % 4
            pts = []
            for mp_ in range(4):
                pt = pp.tile([128, 2, T], FP8, tag="p")
                for half in range(2):
                    m = 2 * mp_ + half
                    ps_sc = ps_mm.tile([128, T], FP32, tag="mm")
                    for nb in range(2):
                        s = slice(512 * nb, 512 * (nb + 1))
                        nc.tensor.matmul(ps_sc[:, s],
                                         kg[g][32 * i:32 * i + 32, :, 128 * m:128 * (m + 1)],
                                         qg[g][32 * i:32 * i + 32, :, s],
                                         start=True, stop=True, perf_mode=DRM,
                                         tile_position=(32 * i, 0))
                    nc.scalar.activation(pt[:, half, :], ps_sc, AF.Exp,
                                         scale=float(HD) ** -0.5)
                pts.append(pt)
            po = ps_aux.tile([66, T], FP32, tag="aux")
            for nb in range(2):
                s = slice(512 * nb, 512 * (nb + 1))
                for mp_ in range(4):
                    nc.tensor.matmul(po[:, s], vT[mp_][:, :, h, :], pts[mp_][:, :, s],
                                     start=(mp_ == 0), stop=(mp_ == 3),
                                     perf_mode=DRM, skip_group_check=True)
            if pend is not None:
                normalize_head(at_tiles, pend[0], pend[1], act_copy=True)
            pend = (h, po)
        normalize_head(at_tiles, pend[0], pend[1], act_copy=True)
        return at_tiles

    teT = []
    for k in range(TD // 128):
        tt = pools["tep"].tile([128, TLP], BF16, tag="teT")
        nc.sync.dma_start(out=tt, in_=io["teT"][128 * k:128 * (k + 1), :])
        teT.append(tt)
    b_ctxT = sb.tile([128, 8], FP32, tag="b_ctxT")
    nc.sync.dma_start(out=b_ctxT, in_=io["b_ctxT"])
    ctx2 = [pools["ctxp"].tile([128, 2, TLP], FP8, tag="ctxT", name=f"ctx2_{i}")
            for i in range(4)]
    for mb in range(4):
        wt = load_wblock(io["w_ctx"], BF16, mb, wst)
        for mm in range(2):
            m = 2 * mb + mm
            pm = ps_aux.tile([128, TLP], FP32, tag="aux")
            for k in range(TD // 128):
                nc.tensor.matmul(pm, wt[:, k, 128 * mm:128 * (mm + 1)], teT[k],
                                 start=(k == 0), stop=(k == TD // 128 - 1))
            nc.vector.tensor_scalar(ctx2[m // 2][:, m % 2, :], pm, b_ctxT[:, m:m + 1],
                                    None, op0=ALU.add)

    kg2 = [qkp.tile([128, 2, TLP], FP8, tag="qk", name=f"kg2_{g}") for g in range(2)]
    for mb in range(2):
        wt = load_wblock(io["w_k"], FP8, mb, wst)
        for mm in range(2):
            c = 2 * mb + mm
            g, b = c // 2, c % 2
            pm = ps_aux.tile([128, TLP], FP32, tag="aux")
            for k2 in range(4):
                nc.tensor.matmul(pm, wt[:, 2 * k2:2 * k2 + 2, 128 * mm:128 * (mm + 1)],
                                 ctx2[k2], start=(k2 == 0), stop=(k2 == 3),
                                 perf_mode=DRM)
            nc.scalar.copy(kg2[g][:, b, :], pm)
    # vc: [80, HL, 65] fp8
    wv2 = [load_wblock(io["w_v"], FP8, g, wst) for g in range(2)]
    pv = ps_aux.tile([TLP, DL], FP32, tag="aux")
    for g in range(2):
        for k2 in range(4):
            nc.tensor.matmul(pv[:, 256 * g:256 * (g + 1)], ctx2[k2],
                             wv2[g][:, 2 * k2:2 * k2 + 2, :],
                             start=(k2 == 0), stop=(k2 == 3), perf_mode=DRM)
    vc = vtp.tile([128, HL, HD + 2], FP8, tag="vt", name="vc")
    nc.vector.memset(vc, 0.0)
    nc.vector.tensor_copy(vc[0:TL, :, 0:HD], pv[0:TL].rearrange("p (a b) -> p a b", a=HL))
    nc.vector.memset(vc[0:TL, :, HD:HD + 1], 1.0)


    def _ada_filler(h):
        if h == 2:
            ada_blocks(range(4, 8))
        elif h == 4:
            ada_blocks(range(8, 12))
        elif h == 6:
            mod_reduce(2048, 6144, 2, 6)

    at1 = attention_self(qg, kg, vT, filler=_ada_filler)

    # ---------------- row-parallel + fold residual + AllReduce ----------------
    def row_parallel_reduce(w_name, bT_name, act_tiles, x_tiles, nk, ar_tag):
        bT = sb.tile([128, 8], FP32, tag=bT_name)
        nc.sync.dma_start(out=bT, in_=io[bT_name])
        ar_in = dram.tile([D, T], BF16, tag=ar_tag + "_i")
        ar_out = dram.tile([D, T], BF16, tag=ar_tag + "_o")
        for mb in range(2):
            wt = load_wblock(io[w_name], BF16, mb, wst2)
            for mm in range(4):
                m = 4 * mb + mm
                pm = ps_mm.tile([128, T], FP32, tag="mm")
                for nb in range(2):
                    s = slice(512 * nb, 512 * (nb + 1))
                    for k in range(nk):
                        nc.tensor.matmul(pm[:, s], wt[:, k, 128 * mm:128 * (mm + 1)],
                                         act_tiles[k][:, s], start=(k == 0),
                                         stop=(k == nk - 1))
                e2 = scr.tile([128, T], BF16, tag="t1")
                nc.vector.tensor_scalar(e2, pm, bT[:, m:m + 1], None, op0=ALU.add)
                nc.sync.dma_start(out=ar_in[128 * m:128 * (m + 1), :], in_=e2)
        newx = []
        for hh in range(4):
            sl = slice(hh * D // 4, (hh + 1) * D // 4)
            if nocc:
                nc.sync.dma_start(out=ar_out[sl, :], in_=ar_in[sl, :])
            else:
                nc.gpsimd.collective_compute(
                    "AllReduce", ALU.add, replica_groups=REPLICA_GROUPS,
                    ins=[ar_in[sl, :].opt()], outs=[ar_out[sl, :].opt()])
            for mm in range(2):
                m = 2 * hh + mm
                bt = scr.tile([128, T], BF16, tag="t1", name=f"arb_{ar_tag}_{m}")
                nc.gpsimd.dma_start(out=bt, in_=ar_out[128 * m:128 * (m + 1), :])
                xt = xp.tile([128, T], FP32R, tag="x")
                nc.vector.tensor_tensor(xt, x_tiles[m], bt, op=ALU.add)
                newx.append(xt)
        return newx

    x2 = row_parallel_reduce("w_proj", "b_projT", at1, x1, 4, "arp")

    # ---------------- Stage 4: cross-attention ----------------
    h2 = layer_norm_mod(x2, g_sh=2, g_sc=1)

    qg2 = [qkp.tile([128, 2, T], FP8, tag="qk", name=f"qg2_{g}") for g in range(2)]
    for mb in range(2):
        wt = load_wblock(io["w_q"], FP8, mb, wst)
        for mm in range(2):
            c = 2 * mb + mm
            g, b = c // 2, c % 2
            pm = ps_mm.tile([128, T], FP32, tag="mm")
            for nb in range(2):
                s = slice(512 * nb, 512 * (nb + 1))
                for k2 in range(4):
                    nc.tensor.matmul(pm[:, s], wt[:, 2 * k2:2 * k2 + 2, 128 * mm:128 * (mm + 1)],
                                     h2[k2][:, :, s], start=(k2 == 0), stop=(k2 == 3),
                                     perf_mode=DRM)
            nc.scalar.copy(qg2[g][:, b, :], pm)
    def attention_cross(qg2, kg2, vc):
        at_tiles = [atp.tile([128, T], BF16, tag="at", name=f"at2_{i}") for i in range(4)]
        pend = None
        for h in range(HL):
            g, i = h // 4, h % 4
            ps_sc = ps_mm.tile([128, T], FP32, tag="mm")
            for nb in range(2):
                s = slice(512 * nb, 512 * (nb + 1))
                nc.tensor.matmul(ps_sc[0:TLP, s], kg2[g][32 * i:32 * i + 32, :, :],
                                 qg2[g][32 * i:32 * i + 32, :, s],
                                 start=True, stop=True, perf_mode=DRM,
                                 tile_position=(32 * i, 0))
            pt = pp.tile([128, T], FP8, tag="p", name=f"p2_{h}")
            nc.scalar.activation(pt[0:TLP, :], ps_sc[0:TLP, :], AF.Exp,
                                 scale=float(HD) ** -0.5)
            po = ps_aux.tile([66, T], FP32, tag="aux")
            for nb in range(2):
                s = slice(512 * nb, 512 * (nb + 1))
                nc.tensor.matmul(po[:, s], vc[0:TLP, h, :], pt[0:TLP, s],
                                 start=True, stop=True, skip_group_check=True)
            if pend is not None:
                normalize_head(at_tiles, pend[0], pend[1], act_copy=True)
            pend = (h, po)
        normalize_head(at_tiles, pend[0], pend[1], act_copy=True)
        return at_tiles

    at2 = attention_cross(qg2, kg2, vc)
    x3 = row_parallel_reduce("w_out", "b_outT", at2, x2, 4, "aro")

    # ---------------- Stage 5: MLP ----------------
    h3, h3d, h3l = layer_norm_mod(x3, g_sh=4, g_sc=2, lo=True)
    b_fc1T = sb.tile([128, 16], FP32, tag="b_fc1T")
    nc.sync.dma_start(out=b_fc1T, in_=io["b_fc1T"])
    b_fc2T = sb.tile([128, 8], FP32, tag="b_fc2T")
    nc.sync.dma_start(out=b_fc2T, in_=io["b_fc2T"])
    for tb in range(2):
        s = slice(512 * tb, 512 * (tb + 1))
        hid = []
        for mb in range(8):
            wh = load_wblock(io["w_fc1h"], FP8, mb, wst)
            wl = load_wblock(io["w_fc1l"], FP8, mb, wst)
            for mm in range(2):
                m = 2 * mb + mm
                ms = slice(128 * mm, 128 * (mm + 1))
                pm = ps_aux.tile([128, 512], FP32, tag="aux")
                for k2 in range(4):
                    nc.tensor.matmul(pm, wh[:, 2 * k2:2 * k2 + 2, ms],
                                     h3[k2][:, :, s], start=(k2 == 0), stop=False,
                                     perf_mode=DRM)
                for k2 in range(4):
                    nc.tensor.matmul(pm, wh[:, 2 * k2:2 * k2 + 2, ms],
                                     h3l[k2][:, :, s], start=False, stop=False,
                                     perf_mode=DRM)
                for k2 in range(4):
                    nc.tensor.matmul(pm, wl[:, 2 * k2:2 * k2 + 2, ms],
                                     h3d[k2][:, :, s], start=False, stop=(k2 == 3),
                                     perf_mode=DRM)
                ht = hidp.tile([128, 512], BF16, tag="hid")
                nc.scalar.activation(ht, pm, AF.Gelu, bias=b_fc1T[:, m:m + 1])
                hid.append(ht)
        for mb in range(4):
            wt2 = load_wblock(io["w_fc2"], BF16, mb, wst2)
            for mm in range(2):
                m = 2 * mb + mm
                pm = ps_mm.tile([128, 512], FP32, tag="mm")
                for k in range(FFL // 128):
                    nc.tensor.matmul(pm, wt2[:, k, 128 * mm:128 * (mm + 1)], hid[k],
                                     start=(k == 0), stop=(k == FFL // 128 - 1))
                ot = scr.tile([128, 512], BF16, tag="ot")
                nc.vector.tensor_scalar(ot, pm, b_fc2T[:, m:m + 1], None, op0=ALU.add)
                nc.vector.scalar_tensor_tensor(ot, x3[m][:, s], mask, ot,
                                               op0=ALU.mult, op1=ALU.add)
                nc.sync.dma_start(out=io["out_xT"][128 * m:128 * (m + 1), s], in_=ot)


def build(nocc=False):
    nc = bacc.Bacc("TRN2", target_bir_lowering=False, debug=False,
                   num_devices=1 if nocc else 8)
    io = _declare(nc)
    with tile.TileContext(nc) as tc:
        import contextlib
        with contextlib.ExitStack() as ctx:
            def pool(name, bufs, space="SBUF"):
                return ctx.enter_context(tc.tile_pool(name=name, bufs=bufs, space=space))
            pools = {
                "sb": pool("sb", 1),
                "xp": pool("xp", 9),
                "hp": pool("hp", 12),
                "qkp": pool("qkp", 6),
                "vtp": pool("vtp", 5),
                "atp": pool("atp", 4),
                "pp": pool("pp", 6),
                "wst": pool("wst", 4),
                "wst2": pool("wst2", 2),
                "hidp": pool("hidp", 16),
                "xsqp": pool("xsqp", 2),
                "scr": pool("scr", 3),
                "rcpp": pool("rcpp", 2),
                "tep": pool("tep", 6),
                "ctxp": pool("ctxp", 4),
                "vecp": pool("vecp", 1),
                "ps_mm": pool("ps_mm", 2, "PSUM"),
                "ps_aux": pool("ps_aux", 2, "PSUM"),
                "dram": pool("dram", 1, "DRAM"),
            }
            _emit(tc, io, pools, nocc=nocc)
    nc.compile()
    return nc


def pretile(w, mblk):
    """[K, M] -> [M//mblk, 128, K//128, mblk] contiguous blocks."""
    K, M = w.shape
    kc = K // 128
    v = w.reshape(kc, 128, M // mblk, mblk).transpose(2, 1, 0, 3)
    return np.ascontiguousarray(v)


def _perm_idx():
    """Column permutation for split-head DR-32 scores: chunk c=2g+b holds
    dims [32b:32b+32) of heads 4g..4g+3."""
    idx = []
    for c in range(4):
        g, b = c // 2, c % 2
        for i in range(4):
            h = 4 * g + i
            idx.extend(range(h * 64 + 32 * b, h * 64 + 32 * b + 32))
    return np.array(idx)


PERM = _perm_idx()


def shard_inputs(inputs):
    f32 = np.float32
    bf16 = ml_dtypes.bfloat16
    f8 = ml_dtypes.float8_e4m3
    x = np.asarray(inputs["x"], f32)
    c = np.asarray(inputs["c"], f32)
    te = np.asarray(inputs["text_embed"], f32)
    W_ada, b_ada = np.asarray(inputs["W_ada"], f32), np.asarray(inputs["b_ada"], f32)
    W_qkv, b_qkv = np.asarray(inputs["W_qkv"], f32), np.asarray(inputs["b_qkv"], f32)
    W_proj, b_proj = np.asarray(inputs["W_proj"], f32), np.asarray(inputs["b_proj"], f32)
    W_ctx, b_ctx = np.asarray(inputs["W_ctx"], f32), np.asarray(inputs["b_ctx"], f32)
    W_q, W_k, W_v = (np.asarray(inputs[k], f32) for k in ("W_q", "W_k", "W_v"))
    W_out, b_out = np.asarray(inputs["W_out"], f32), np.asarray(inputs["b_out"], f32)
    W_fc1, b_fc1 = np.asarray(inputs["W_fc1"], f32), np.asarray(inputs["b_fc1"], f32)
    W_fc2, b_fc2 = np.asarray(inputs["W_fc2"], f32), np.asarray(inputs["b_fc2"], f32)

    maps = []
    for core in range(8):
        b, hf = core // 2, core % 2
        sl = slice(DL * hf, DL * (hf + 1))
        half = (lambda a: a) if hf == 0 else (lambda a: np.zeros_like(a))
        qs = slice(DL * hf, DL * (hf + 1))
        ks_ = slice(D + DL * hf, D + DL * (hf + 1))
        vs = slice(2 * D + DL * hf, 2 * D + DL * (hf + 1))
        wq_p = W_qkv[:, qs][:, PERM]
        wk_p = W_qkv[:, ks_][:, PERM]
        w1 = W_fc1[:, FFL * hf:FFL * (hf + 1)]
        w1h = w1.astype(f8)
        w1l = ((w1 - w1h.astype(f32)) * 64).astype(f8)
        m = {
            "xT": np.ascontiguousarray(x[b].T).astype(bf16),
            "cT": np.ascontiguousarray(c[b, sl].reshape(4, 128).T),
            "teT": np.ascontiguousarray(np.pad(te[b].T, ((0, 0), (0, TLP - TL)))).astype(bf16),
            "w_ada": pretile(W_ada[sl, :].astype(bf16), 512),
            "b_adaT": np.ascontiguousarray(b_ada.reshape(6, 8, 128).transpose(2, 0, 1)),
            "w_qkv": pretile(np.concatenate(
                [wq_p, wk_p, W_qkv[:, vs]], axis=1).astype(f8), 256),
            "b_qkT": np.ascontiguousarray(
                np.concatenate([b_qkv[qs][PERM], b_qkv[ks_][PERM]]).reshape(8, 128).T),
            "b_v": b_qkv[vs][None, :],
            "w_proj": pretile(W_proj[sl, :].astype(bf16), 512),
            "b_projT": np.ascontiguousarray(half(b_proj).reshape(8, 128).T),
            "w_ctx": pretile(W_ctx.astype(bf16), 256),
            "b_ctxT": np.ascontiguousarray(b_ctx.reshape(8, 128).T),
            "w_q": pretile(W_q[:, sl][:, PERM].astype(f8), 256),
            "w_k": pretile(W_k[:, sl][:, PERM].astype(f8), 256),
            "w_v": pretile(W_v[:, sl].astype(f8), 256),
            "w_out": pretile(W_out[sl, :].astype(bf16), 512),
            "b_outT": np.ascontiguousarray(half(b_out).reshape(8, 128).T),
            "w_fc1h": pretile(w1h, 256),
            "w_fc1l": pretile(w1l, 256),
            "b_fc1T": np.ascontiguousarray(
                b_fc1[FFL * hf:FFL * (hf + 1)].reshape(16, 128).T),
            "w_fc2": pretile(W_fc2[FFL * hf:FFL * (hf + 1), :].astype(bf16), 256),
            "b_fc2T": np.ascontiguousarray(half(b_fc2).reshape(8, 128).T),
            "ones_r": np.ones((128, 128), f32),
            "ones_b": np.ones((128, 8), bf16),
            "maskT": np.full((128, 1), 1.0 - hf, f32),
        }
        maps.append(m)
    return maps


_NC_CACHE = None


def kernel(**inputs):
    global _NC_CACHE
    if _NC_CACHE is None:
        _NC_CACHE = build()
    nc = _NC_CACHE
    in_maps = shard_inputs(inputs)
    res = run_bass_kernel_spmd(nc, in_maps, core_ids=list(range(8)))
    out = np.empty((B, N, D), np.float32)
    for b in range(B):
        p0 = res.results[2 * b]["out_xT"]
        p1 = res.results[2 * b + 1]["out_xT"]
        out[b] = (p0.astype(np.float32) + p1.astype(np.float32)).T
    return out


# revision 59
# speedup vs baseline: 1.0146x; 1.0146x over previous
"""DiT block kernel for TRN2, 8 NeuronCores.

Sharding: DP=4 over batch x TP=2 over heads (Megatron style).
Core c: batch b=c//2, half hf=c%2 (8 of 16 heads, 2048/4096 MLP cols, 512/1024
rows of the row-parallel weights).

Device layout is feature-major: activations [feature(partitions), token(free)], so
every matmul chains without transposes: outT = W.T @ actT with lhsT=W natural layout.
v^T (lhsT of attn@v) is produced by swapping matmul operands. Softmax runs without
max-subtraction (scores are O(1)); denominators come from an augmented ones-column in
v^T (row 64 of the [65, tok] attention output).

Dtypes (cost model: fp8e4+DoubleRow matmul = 0.5 cyc/row over TWO k-tiles = 4x bf16
MAC rate; DVE 2x for all-16-bit ops; DMA is a serialized shared device):
 - fp8e4 + DoubleRow: qkv, v, self-attn scores (32-partition split-head layout),
   attn@v, q2/kc/vc, fc1 (3 DR terms: h8@W8 + (h3-h8)@W8 + (h3/64)@(64*(W-W8)) --
   hi/lo splits on BOTH operands keep fc1 at bf16-level accuracy).
 - bf16: W_ada, W_ctx/te, proj, out, fc2, AllReduce buffers (branch-only), hidden, at.
 - fp32: trunk x (residuals accumulate locally; AllReduce carries branch partials
   only), PSUM, LN stats, mod.

Self-attn q/k use a permuted column layout: chunk c=2g+b holds dims [32b:32b+32)
of heads 4g..4g+3, so scores contract HD=64 as a DoubleRow pair of 32-partition
tiles at base partitions {0,32,64,96}.


Comms: 3 pair AllReduces (adaLN mod fp32, self-attn proj+residual bf16, cross-attn
out+residual bf16); the fc2 reduce is folded into the output (host adds partials).
"""
import sys
import numpy as np

sys.path.insert(0, "/opt/trn_rl_repo")

import ml_dtypes
import concourse.bass as bass
import concourse.mybir as mybir
import concourse.tile as tile
from concourse import bacc
from concourse.bass_utils import run_bass_kernel_spmd

FP32 = mybir.dt.float32
FP32R = mybir.dt.float32r
BF16 = mybir.dt.bfloat16
FP8 = mybir.dt.float8e4
AF = mybir.ActivationFunctionType
ALU = mybir.AluOpType
DRM = mybir.MatmulPerfMode.DoubleRow
I32 = mybir.dt.int32
EXP_A = 12102203.161561485   # 2^23 / ln2 (Schraudolph fast-exp)
EXP_B = 1064985216.0         # 127*2^23 - 368000, exactly representable in fp32

B, N, D, H, TD, TL = 4, 1024, 1024, 16, 768, 77
HD = 64
EPS = 1e-6
HL = 8
DL = 512
FFL = 2048
T = 1024
TLP = 80
NCH = D // 128
REPLICA_GROUPS = [[0, 1], [2, 3], [4, 5], [6, 7]]


def _declare(nc):
    d = {}

    def inp(name, shape, dt):
        d[name] = nc.dram_tensor(name, list(shape), dt, kind="ExternalInput").ap()

    inp("xT", (D, T), BF16)
    inp("cT", (128, 4), FP32)
    inp("teT", (TD, TLP), BF16)
    inp("w_ada", (12, 128, 4, 512), BF16)
    inp("b_adaT", (128, 6, 8), FP32)
    inp("w_qkv", (6, 128, 8, 256), FP8)
    inp("b_qkT", (128, 8), FP32)
    inp("b_v", (1, DL), FP32R)
    inp("w_proj", (2, 128, 4, 512), BF16)
    inp("b_projT", (128, 8), FP32)
    inp("w_ctx", (4, 128, 6, 256), BF16)
    inp("b_ctxT", (128, 8), FP32)
    inp("w_q", (2, 128, 8, 256), FP8)
    inp("w_k", (2, 128, 8, 256), FP8)
    inp("w_v", (2, 128, 8, 256), FP8)
    inp("w_out", (2, 128, 4, 512), BF16)
    inp("b_outT", (128, 8), FP32)
    inp("w_fc1h", (8, 128, 8, 256), FP8)
    inp("w_fc1l", (8, 128, 8, 256), FP8)
    inp("b_fc1T", (128, 16), FP32)
    inp("w_fc2", (4, 128, 16, 256), BF16)
    inp("b_fc2T", (128, 8), FP32)
    inp("ones_r", (128, 128), FP32R)
    inp("ones_b", (128, 8), BF16)
    inp("maskT", (128, 1), FP32)
    d["out_xT"] = nc.dram_tensor("out_xT", [D, T], BF16, kind="ExternalOutput").ap()
    return d


def _emit(tc, io, pools, nocc=False):
    nc = tc.nc
    sb = pools["sb"]
    xp, hp, qkp, vtp, atp, pp = (pools[k] for k in ("xp", "hp", "qkp", "vtp", "atp", "pp"))
    wst, wst2, hidp, xsqp, scr, rcpp = (pools[k] for k in
                                        ("wst", "wst2", "hidp", "xsqp", "scr", "rcpp"))
    vecp = pools["vecp"]
    ps_mm, ps_aux = pools["ps_mm"], pools["ps_aux"]
    dram = pools["dram"]

    ones = sb.tile([128, 128], FP32R, tag="ones")
    nc.sync.dma_start(out=ones, in_=io["ones_r"])
    ones_b = sb.tile([128, 8], BF16, tag="ones_b")
    nc.sync.dma_start(out=ones_b, in_=io["ones_b"])
    mask = sb.tile([128, 1], FP32, tag="mask")
    nc.sync.dma_start(out=mask, in_=io["maskT"])

    def load_wblock(src_ap, dt, bi, pool):
        _, _, kc, mblk = src_ap.shape
        wt = pool.tile([128, kc, mblk], dt, tag="w")
        nc.sync.dma_start(out=wt, in_=src_ap[bi])
        return wt

    # ---------------- Stage 0: adaLN ----------------
    cT = sb.tile([128, 4], FP32, tag="cT")
    nc.sync.dma_start(out=cT, in_=io["cT"])
    cs = sb.tile([128, 4], BF16, tag="cs")
    nc.scalar.activation(cs, cT, AF.Silu)

    ar_mod_in = dram.tile([1, 6 * D], BF16, tag="armod_i")
    ar_mod_out = dram.tile([1, 6 * D], BF16, tag="armod_o")

    b_adaT = sb.tile([128, 6, 8], FP32, tag="b_adaT")
    nc.sync.dma_start(out=b_adaT, in_=io["b_adaT"])
    modT = sb.tile([128, 6, 8], FP32, tag="modT")
    ksc = sb.tile([128, 3, 8], FP32, tag="ksc")
    mod_view = ar_mod_out.rearrange("o (g j p) -> (o p) g j", p=128, g=6)

    def ada_blocks(mbs):
        for mb in mbs:   # w_ada rhs-blocks [512, 512]
            wt = load_wblock(io["w_ada"], BF16, mb, wst)
            pm = (ps_aux if mb % 2 == 0 else ps_mm).tile([1, 512], FP32,
                                                         tag="aux" if mb % 2 == 0 else "mm")
            for k in range(4):
                nc.tensor.matmul(pm, cs[:, k:k + 1], wt[:, k, :],
                                 start=(k == 0), stop=(k == 3))
            mp = vecp.tile([1, 512], BF16, tag="tmpv", name=f"modp_{mb}")
            nc.vector.tensor_copy(mp, pm)
            nc.sync.dma_start(out=ar_mod_in[:, 512 * mb:512 * (mb + 1)], in_=mp)

    def mod_reduce(c0, c1, g0, g1):
        if nocc:
            nc.sync.dma_start(out=ar_mod_out[:, c0:c1], in_=ar_mod_in[:, c0:c1])
        else:
            nc.gpsimd.collective_compute(
                "AllReduce", ALU.add, replica_groups=REPLICA_GROUPS,
                ins=[ar_mod_in[:, c0:c1].opt()], outs=[ar_mod_out[:, c0:c1].opt()])
        mtmp = sb.tile([128, 6, 8], BF16, tag="mtmp", name=f"mtmp{g0}")
        nc.sync.dma_start(out=mtmp[:, g0:g1, :], in_=mod_view[:, g0:g1, :])
        nc.vector.tensor_tensor(modT[:, g0:g1, :], mtmp[:, g0:g1, :],
                                b_adaT[:, g0:g1, :], op=ALU.add)
        for i in range(g0 // 2, g1 // 2):
            nc.vector.tensor_scalar(ksc[:, i, :], modT[:, 2 * i + 1, :], 1.0, None,
                                    op0=ALU.add)

    ada_blocks(range(4))
    mod_reduce(0, 2048, 0, 2)
    eps_t = sb.tile([1, 1], FP32, tag="eps")
    nc.vector.memset(eps_t, EPS)

    # ---------------- LN + modulate -> fp8 h pairs ----------------
    def layer_norm_mod(x_tiles, g_sh, g_sc, lo=False):
        bf_x = g_sh == 0
        lones = ones_b if bf_x else ones
        sum_ps = ps_aux.tile([1, T], FP32, tag="aux")
        sq_ps = ps_aux.tile([1, T], FP32, tag="aux")
        for j in range(NCH):
            xsq = xsqp.tile([128, T], BF16 if bf_x else FP32R, tag="xsq")
            if bf_x:
                nc.vector.tensor_tensor(xsq, x_tiles[j], x_tiles[j], op=ALU.mult)
            else:
                nc.scalar.activation(xsq, x_tiles[j], AF.Square)
            for nb in range(2):
                s = slice(512 * nb, 512 * (nb + 1))
                nc.tensor.matmul(sum_ps[:, s], lones[:, 0:1], x_tiles[j][:, s],
                                 start=(j == 0), stop=(j == NCH - 1), skip_group_check=True)
                nc.tensor.matmul(sq_ps[:, s], lones[:, 1:2], xsq[:, s],
                                 start=(j == 0), stop=(j == NCH - 1), skip_group_check=True)
        mu = vecp.tile([1, T], FP32R, tag="mu")
        nc.scalar.activation(mu, sum_ps, AF.Copy, scale=1.0 / D)
        musq = vecp.tile([1, T], FP32, tag="tmpv")
        nc.vector.tensor_tensor(musq, mu, mu, op=ALU.mult)
        var = vecp.tile([1, T], FP32, tag="tmpv2")
        nc.vector.scalar_tensor_tensor(var, sq_ps, 1.0 / D, musq,
                                       op0=ALU.mult, op1=ALU.subtract)
        sig = vecp.tile([1, T], FP32, tag="tmpv")
        nc.scalar.activation(sig, var, AF.Sqrt, bias=eps_t)
        rsig = vecp.tile([1, T], FP32R, tag="rsig")
        with nc.allow_low_precision(reason="fp32r rsig feeds fp32r broadcast matmul"):
            nc.vector.reciprocal(rsig, sig)
        rs_b = ps_aux.tile([128, T], FP32, tag="aux")
        mrs_b = ps_aux.tile([128, T], FP32, tag="aux")
        for nb in range(2):
            s = slice(512 * nb, 512 * (nb + 1))
            nc.tensor.matmul(rs_b[:, s], ones[0:1, :], rsig[:, s], start=True, stop=True)
        nmrs = vecp.tile([1, T], FP32R, tag="tmpv", name="nmrs")
        with nc.allow_low_precision(reason="fp32r nmrs feeds fp32r broadcast matmul"):
            nc.vector.scalar_tensor_tensor(nmrs, mu, -1.0, rsig,
                                           op0=ALU.mult, op1=ALU.mult)
        rs_sb = xsqp.tile([128, T], BF16, tag="xsq", name=f"rs_sb{g_sh}")
        nc.scalar.copy(rs_sb, rs_b)
        for nb in range(2):
            s = slice(512 * nb, 512 * (nb + 1))
            nc.tensor.matmul(mrs_b[:, s], ones[0:1, :], nmrs[:, s], start=True, stop=True)
        mrs_sb = xsqp.tile([128, T], BF16, tag="xsq", name=f"mrs_sb{g_sh}")
        nc.scalar.copy(mrs_sb, mrs_b)
        h_tiles = [hp.tile([128, 2, T], FP8, tag="h", name=f"h{g_sh}_{p}")
                   for p in range(4)]
        hd_tiles = hl_tiles = None
        if lo:
            hd_tiles = [hp.tile([128, 2, T], FP8, tag="h", name=f"hd_{p}")
                        for p in range(4)]
            hl_tiles = [hp.tile([128, 2, T], FP8, tag="h", name=f"hl_{p}")
                        for p in range(4)]
            kscd = sb.tile([128, 8], FP32, tag="kscd")
            nc.vector.tensor_scalar(kscd, ksc[:, g_sc, :], 1.0 / 64, None, op0=ALU.mult)
            shd = sb.tile([128, 8], FP32, tag="shd")
            nc.vector.tensor_scalar(shd, modT[:, g_sh, :], 1.0 / 64, None, op0=ALU.mult)

        for j in range(NCH):
            u = scr.tile([128, T], BF16, tag="t1")
            nc.vector.tensor_tensor(u, x_tiles[j], rs_sb, op=ALU.mult)
            nc.vector.tensor_tensor(u, u, mrs_sb, op=ALU.add)
            h8 = h_tiles[j // 2][:, j % 2, :]
            nc.scalar.activation(h8, u, AF.Identity,
                                 scale=ksc[:, g_sc, j:j + 1],
                                 bias=modT[:, g_sh, j:j + 1])
            if lo:
                nc.scalar.activation(hd_tiles[j // 2][:, j % 2, :], u, AF.Identity,
                                     scale=kscd[:, j:j + 1], bias=shd[:, j:j + 1])
                h3s = scr.tile([128, T], FP32, tag="h3s")
                nc.scalar.activation(h3s, u, AF.Identity,
                                     scale=ksc[:, g_sc, j:j + 1],
                                     bias=modT[:, g_sh, j:j + 1])
                nc.vector.scalar_tensor_tensor(hl_tiles[j // 2][:, j % 2, :],
                                               h8, -1.0, h3s,
                                               op0=ALU.mult, op1=ALU.add)
        if lo:
            return h_tiles, hd_tiles, hl_tiles
        return h_tiles

    # ---------------- Stage 1: x + LN1 ----------------
    x1 = []
    for j in range(NCH):
        xt = xp.tile([128, T], BF16, tag="x")
        nc.sync.dma_start(out=xt, in_=io["xT"][128 * j:128 * (j + 1), :])
        x1.append(xt)
    h1 = layer_norm_mod(x1, g_sh=0, g_sc=0)

    # ---------------- Stage 2: qkv + vT ----------------
    b_qkT = sb.tile([128, 8], FP32, tag="b_qkT")
    nc.sync.dma_start(out=b_qkT, in_=io["b_qkT"])
    bvr = sb.tile([1, DL], FP32R, tag="bv")
    nc.sync.dma_start(out=bvr, in_=io["b_v"])
    # q,k: split-head layout [128=4heads x 32dims, 2 halves, T], fp8 DoubleRow
    qg = [qkp.tile([128, 2, T], FP8, tag="qk", name=f"qg{g}") for g in range(2)]
    kg = [qkp.tile([128, 2, T], FP8, tag="qk", name=f"kg{g}") for g in range(2)]
    for mb in range(4):
        wt = load_wblock(io["w_qkv"], FP8, mb, wst)
        for mm in range(2):
            c = 2 * mb + mm
            pm = ps_mm.tile([128, T], FP32, tag="mm")
            for nb in range(2):
                s = slice(512 * nb, 512 * (nb + 1))
                for k2 in range(4):
                    nc.tensor.matmul(pm[:, s], wt[:, 2 * k2:2 * k2 + 2, 128 * mm:128 * (mm + 1)],
                                     h1[k2][:, :, s], start=(k2 == 0), stop=(k2 == 3),
                                     perf_mode=DRM)
            tgt = qg if c < 4 else kg
            g, b = (c % 4) // 2, c % 2
            nc.scalar.activation(tgt[g][:, b, :], pm, AF.Identity,
                                 bias=b_qkT[:, c:c + 1])
    # vT: token-major pairs [128, 2, HL, HD+1] with ones-augmented head columns
    wv_blk = [load_wblock(io["w_qkv"], FP8, 4 + g, wst) for g in range(2)]
    vT = []
    for mp_ in range(4):
        vt = vtp.tile([128, 2, HL, HD + 2], FP8, tag="vt")
        for half in range(2):
            m = 2 * mp_ + half
            pm = ps_aux.tile([128, DL], FP32, tag="aux")
            for g in range(2):
                for k2 in range(4):
                    nc.tensor.matmul(pm[:, 256 * g:256 * (g + 1)],
                                     h1[k2][:, :, 128 * m:128 * (m + 1)],
                                     wv_blk[g][:, 2 * k2:2 * k2 + 2, :],
                                     start=(k2 == 0), stop=False, perf_mode=DRM)
                nc.tensor.matmul(pm[:, 256 * g:256 * (g + 1)], ones[0:1, :],
                                 bvr[:, 256 * g:256 * (g + 1)], start=False, stop=(g == 1),
                                 skip_group_check=True)
            nc.scalar.copy(vt[:, half, :, 0:HD],
                           pm.rearrange("p (a b) -> p a b", a=HL))
            nc.vector.memset(vt[:, half, :, HD:HD + 1], 1.0)
        vT.append(vt)

    # ---------------- self-attention (fp8, DR scores + DR av) ----------------
    def normalize_head(at_tiles, h, po, act_copy=False):
        ti, off = h // 2, 64 * (h % 2)
        rcp = rcpp.tile([1, T], FP32R, tag="rcp")
        with nc.allow_low_precision(reason="fp32r rcp feeds fp32r broadcast matmul"):
            nc.vector.reciprocal(rcp, po[64:65, :])
        pb = ps_mm.tile([64, T], FP32, tag="mm")
        for nb in range(2):
            s = slice(512 * nb, 512 * (nb + 1))
            nc.tensor.matmul(pb[:, s], ones[0:1, 0:64], rcp[:, s], start=True, stop=True)
        rc_sb = rcpp.tile([64, T], BF16, tag="rc_sb")
        if act_copy:
            nc.scalar.copy(rc_sb, pb)
        else:
            nc.vector.tensor_copy(rc_sb, pb)
        nc.vector.tensor_tensor(at_tiles[ti][off:off + 64, :], po[0:64, :], rc_sb,
                                op=ALU.mult)

    def attention_self(qg, kg, vT, filler=None):
        at_tiles = [atp.tile([128, T], BF16, tag="at", name=f"at_{i}") for i in range(4)]
        pend = None
        for h in range(HL):
            if filler is not None:
                filler(h)
            g, i = h // 4, h % 4
            pts = []
            for mp_ in range(4):
                pt = pp.tile([128, 2, T], FP8, tag="p")
                for half in range(2):
                    m = 2 * mp_ + half
                    ps_sc = ps_mm.tile([128, T], FP32, tag="mm")
                    for nb in range(2):
                        s = slice(512 * nb, 512 * (nb + 1))
                        nc.tensor.matmul(ps_sc[:, s],
                                         kg[g][32 * i:32 * i + 32, :, 128 * m:128 * (m + 1)],
                                         qg[g][32 * i:32 * i + 32, :, s],
                                         start=True, stop=True, perf_mode=DRM,
                                         tile_position=(32 * i, 0))
                    nc.scalar.activation(pt[:, half, :], ps_sc, AF.Exp,
                                         scale=float(HD) ** -0.5)
                pts.append(pt)
            po = ps_aux.tile([66, T], FP32, tag="aux")
            for nb in range(2):
                s = slice(512 * nb, 512 * (nb + 1))
                for mp_ in range(4):
                    nc.tensor.matmul(po[:, s], vT[mp_][:, :, h, :], pts[mp_][:, :, s],
                                     start=(mp_ == 0), stop=(mp_ == 3),
                                     perf_mode=DRM, skip_group_check=True)
            if pend is not None:
                normalize_head(at_tiles, pend[0], pend[1], act_copy=True)
            pend = (h, po)
        normalize_head(at_tiles, pend[0], pend[1], act_copy=True)
        return at_tiles

    teT = []
    for k in range(TD // 128):
        tt = pools["tep"].tile([128, TLP], BF16, tag="teT")
        nc.sync.dma_start(out=tt, in_=io["teT"][128 * k:128 * (k + 1), :])
        teT.append(tt)
    b_ctxT = sb.tile([128, 8], FP32, tag="b_ctxT")
    nc.sync.dma_start(out=b_ctxT, in_=io["b_ctxT"])
    ctx2 = [pools["ctxp"].tile([128, 2, TLP], FP8, tag="ctxT", name=f"ctx2_{i}")
            for i in range(4)]
    for mb in range(4):
        wt = load_wblock(io["w_ctx"], BF16, mb, wst)
        for mm in range(2):
            m = 2 * mb + mm
            pm = ps_aux.tile([128, TLP], FP32, tag="aux")
            for k in range(TD // 128):
                nc.tensor.matmul(pm, wt[:, k, 128 * mm:128 * (mm + 1)], teT[k],
                                 start=(k == 0), stop=(k == TD // 128 - 1))
            nc.vector.tensor_scalar(ctx2[m // 2][:, m % 2, :], pm, b_ctxT[:, m:m + 1],
                                    None, op0=ALU.add)

    kg2 = [qkp.tile([128, 2, TLP], FP8, tag="qk", name=f"kg2_{g}") for g in range(2)]
    for mb in range(2):
        wt = load_wblock(io["w_k"], FP8, mb, wst)
        for mm in range(2):
            c = 2 * mb + mm
            g, b = c // 2, c % 2
            pm = ps_aux.tile([128, TLP], FP32, tag="aux")
            for k2 in range(4):
                nc.tensor.matmul(pm, wt[:, 2 * k2:2 * k2 + 2, 128 * mm:128 * (mm + 1)],
                                 ctx2[k2], start=(k2 == 0), stop=(k2 == 3),
                                 perf_mode=DRM)
            nc.scalar.copy(kg2[g][:, b, :], pm)
    # vc: [80, HL, 65] fp8
    wv2 = [load_wblock(io["w_v"], FP8, g, wst) for g in range(2)]
    pv = ps_aux.tile([TLP, DL], FP32, tag="aux")
    for g in range(2):
        for k2 in range(4):
            nc.tensor.matmul(pv[:, 256 * g:256 * (g + 1)], ctx2[k2],
                             wv2[g][:, 2 * k2:2 * k2 + 2, :],
                             start=(k2 == 0), stop=(k2 == 3), perf_mode=DRM)
    vc = vtp.tile([128, HL, HD + 2], FP8, tag="vt", name="vc")
    nc.vector.memset(vc, 0.0)
    nc.vector.tensor_copy(vc[0:TL, :, 0:HD], pv[0:TL].rearrange("p (a b) -> p a b", a=HL))
    nc.vector.memset(vc[0:TL, :, HD:HD + 1], 1.0)


    def _ada_filler(h):
        if h == 2:
            ada_blocks(range(4, 8))
        elif h == 4:
            ada_blocks(range(8, 12))
        elif h == 6:
            mod_reduce(2048, 6144, 2, 6)

    at1 = attention_self(qg, kg, vT, filler=_ada_filler)

    # ---------------- row-parallel + fold residual + AllReduce ----------------
    def row_parallel_reduce(w_name, bT_name, act_tiles, x_tiles, nk, ar_tag):
        bT = sb.tile([128, 8], FP32, tag=bT_name)
        nc.sync.dma_start(out=bT, in_=io[bT_name])
        ar_in = dram.tile([D, T], BF16, tag=ar_tag + "_i")
        ar_out = dram.tile([D, T], BF16, tag=ar_tag + "_o")
        for mb in range(2):
            wt = load_wblock(io[w_name], BF16, mb, wst2)
            for mm in range(4):
                m = 4 * mb + mm
                pm = ps_mm.tile([128, T], FP32, tag="mm")
                for nb in range(2):
                    s = slice(512 * nb, 512 * (nb + 1))
                    for k in range(nk):
                        nc.tensor.matmul(pm[:, s], wt[:, k, 128 * mm:128 * (mm + 1)],
                                         act_tiles[k][:, s], start=(k == 0),
                                         stop=(k == nk - 1))
                e2 = scr.tile([128, T], BF16, tag="t1")
                nc.vector.tensor_scalar(e2, pm, bT[:, m:m + 1], None, op0=ALU.add)
                nc.sync.dma_start(out=ar_in[128 * m:128 * (m + 1), :], in_=e2)
        newx = []
        for hh in range(4):
            sl = slice(hh * D // 4, (hh + 1) * D // 4)
            if nocc:
                nc.sync.dma_start(out=ar_out[sl, :], in_=ar_in[sl, :])
            else:
                nc.gpsimd.collective_compute(
                    "AllReduce", ALU.add, replica_groups=REPLICA_GROUPS,
                    ins=[ar_in[sl, :].opt()], outs=[ar_out[sl, :].opt()])
            for mm in range(2):
                m = 2 * hh + mm
                bt = scr.tile([128, T], BF16, tag="t1", name=f"arb_{ar_tag}_{m}")
                nc.gpsimd.dma_start(out=bt, in_=ar_out[128 * m:128 * (m + 1), :])
                xt = xp.tile([128, T], FP32R, tag="x")
                nc.vector.tensor_tensor(xt, x_tiles[m], bt, op=ALU.add)
                newx.append(xt)
        return newx

    x2 = row_parallel_reduce("w_proj", "b_projT", at1, x1, 4, "arp")

    # ---------------- Stage 4: cross-attention ----------------
    h2 = layer_norm_mod(x2, g_sh=2, g_sc=1)

    qg2 = [qkp.tile([128, 2, T], FP8, tag="qk", name=f"qg2_{g}") for g in range(2)]
    for mb in range(2):
        wt = load_wblock(io["w_q"], FP8, mb, wst)
        for mm in range(2):
            c = 2 * mb + mm
            g, b = c // 2, c % 2
            pm = ps_mm.tile([128, T], FP32, tag="mm")
            for nb in range(2):
                s = slice(512 * nb, 512 * (nb + 1))
                for k2 in range(4):
                    nc.tensor.matmul(pm[:, s], wt[:, 2 * k2:2 * k2 + 2, 128 * mm:128 * (mm + 1)],
                                     h2[k2][:, :, s], start=(k2 == 0), stop=(k2 == 3),
                                     perf_mode=DRM)
            nc.scalar.copy(qg2[g][:, b, :], pm)
    def attention_cross(qg2, kg2, vc):
        at_tiles = [atp.tile([128, T], BF16, tag="at", name=f"at2_{i}") for i in range(4)]
        pend = None
        for h in range(HL):
            g, i = h // 4, h % 4
            ps_sc = ps_mm.tile([128, T], FP32, tag="mm")
            for nb in range(2):
                s = slice(512 * nb, 512 * (nb + 1))
                nc.tensor.matmul(ps_sc[0:TLP, s], kg2[g][32 * i:32 * i + 32, :, :],
                                 qg2[g][32 * i:32 * i + 32, :, s],
                                 start=True, stop=True, perf_mode=DRM,
                                 tile_position=(32 * i, 0))
            pt = pp.tile([128, T], FP8, tag="p", name=f"p2_{h}")
            nc.scalar.activation(pt[0:TLP, :], ps_sc[0:TLP, :], AF.Exp,
                                 scale=float(HD) ** -0.5)
            po = ps_aux.tile([66, T], FP32, tag="aux")
            for nb in range(2):
                s = slice(512 * nb, 512 * (nb + 1))
                nc.tensor.matmul(po[:, s], vc[0:TLP, h, :], pt[0:TLP, s],
                                 start=True, stop=True, skip_group_check=True)
            if pend is not None:
                normalize_head(at_tiles, pend[0], pend[1], act_copy=True)
            pend = (h, po)
        normalize_head(at_tiles, pend[0], pend[1], act_copy=True)
        return at_tiles

    at2 = attention_cross(qg2, kg2, vc)
    x3 = row_parallel_reduce("w_out", "b_outT", at2, x2, 4, "aro")

    # ---------------- Stage 5: MLP ----------------
    h3, h3d, h3l = layer_norm_mod(x3, g_sh=4, g_sc=2, lo=True)
    b_fc1T = sb.tile([128, 16], FP32, tag="b_fc1T")
    nc.sync.dma_start(out=b_fc1T, in_=io["b_fc1T"])
    b_fc2T = sb.tile([128, 8], FP32, tag="b_fc2T")
    nc.sync.dma_start(out=b_fc2T, in_=io["b_fc2T"])
    for tb in range(2):
        s = slice(512 * tb, 512 * (tb + 1))
        hid = []
        for mb in range(8):
            wh = load_wblock(io["w_fc1h"], FP8, mb, wst)
            wl = load_wblock(io["w_fc1l"], FP8, mb, wst)
            for mm in range(2):
                m = 2 * mb + mm
                ms = slice(128 * mm, 128 * (mm + 1))
                pm = ps_aux.tile([128, 512], FP32, tag="aux")
                for k2 in range(4):
                    nc.tensor.matmul(pm, wh[:, 2 * k2:2 * k2 + 2, ms],
                                     h3[k2][:, :, s], start=(k2 == 0), stop=False,
                                     perf_mode=DRM)
                for k2 in range(4):
                    nc.tensor.matmul(pm, wh[:, 2 * k2:2 * k2 + 2, ms],
                                     h3l[k2][:, :, s], start=False, stop=False,
                                     perf_mode=DRM)
                for k2 in range(4):
                    nc.tensor.matmul(pm, wl[:, 2 * k2:2 * k2 + 2, ms],
                                     h3d[k2][:, :, s], start=False, stop=(k2 == 3),
                                     perf_mode=DRM)
                ht = hidp.tile([128, 512], BF16, tag="hid")
                nc.scalar.activation(ht, pm, AF.Gelu, bias=b_fc1T[:, m:m + 1])
                hid.append(ht)
        for mb in range(4):
            wt2 = load_wblock(io["w_fc2"], BF16, mb, wst2)
            for mm in range(2):
                m = 2 * mb + mm
                pm = ps_mm.tile([128, 512], FP32, tag="mm")
                for k in range(FFL // 128):
                    nc.tensor.matmul(pm, wt2[:, k, 128 * mm:128 * (mm + 1)], hid[k],
                                     start=(k == 0), stop=(k == FFL // 128 - 1))
                ot = scr.tile([128, 512], BF16, tag="ot")
                nc.vector.tensor_scalar(ot, pm, b_fc2T[:, m:m + 1], None, op0=ALU.add)
                nc.vector.scalar_tensor_tensor(ot, x3[m][:, s], mask, ot,
                                               op0=ALU.mult, op1=ALU.add)
                nc.sync.dma_start(out=io["out_xT"][128 * m:128 * (m + 1), s], in_=ot)


def build(nocc=False):
    nc = bacc.Bacc("TRN2", target_bir_lowering=False, debug=False,
                   num_devices=1 if nocc else 8)
    io = _declare(nc)
    with tile.TileContext(nc) as tc:
        import contextlib
        with contextlib.ExitStack() as ctx:
            def pool(name, bufs, space="SBUF"):
                return ctx.enter_context(tc.tile_pool(name=name, bufs=bufs, space=space))
            pools = {
                "sb": pool("sb", 1),
                "xp": pool("xp", 9),
                "hp": pool("hp", 12),
                "qkp": pool("qkp", 6),
                "vtp": pool("vtp", 5),
                "atp": pool("atp", 4),
                "pp": pool("pp", 6),
                "wst": pool("wst", 4),
                "wst2": pool("wst2", 2),
                "hidp": pool("hidp", 16),
                "xsqp": pool("xsqp", 2),
                "scr": pool("scr", 3),
                "rcpp": pool("rcpp", 2),
                "tep": pool("tep", 6),
                "ctxp": pool("ctxp", 4),
                "vecp": pool("vecp", 1),
                "ps_mm": pool("ps_mm", 2, "PSUM"),
                "ps_aux": pool("ps_aux", 2, "PSUM"),
                "dram": pool("dram", 1, "DRAM"),
            }
            _emit(tc, io, pools, nocc=nocc)
    nc.compile()
    return nc


def pretile(w, mblk):
    """[K, M] -> [M//mblk, 128, K//128, mblk] contiguous blocks."""
    K, M = w.shape
    kc = K // 128
    v = w.reshape(kc, 128, M // mblk, mblk).transpose(2, 1, 0, 3)
    return np.ascontiguousarray(v)


def _perm_idx():
    """Column permutation for split-head DR-32 scores: chunk c=2g+b holds
    dims [32b:32b+32) of heads 4g..4g+3."""
    idx = []
    for c in range(4):
        g, b = c // 2, c % 2
        for i in range(4):
            h = 4 * g + i
            idx.extend(range(h * 64 + 32 * b, h * 64 + 32 * b + 32))
    return np.array(idx)


PERM = _perm_idx()


def shard_inputs(inputs):
    f32 = np.float32
    bf16 = ml_dtypes.bfloat16
    f8 = ml_dtypes.float8_e4m3
    x = np.asarray(inputs["x"], f32)
    c = np.asarray(inputs["c"], f32)
    te = np.asarray(inputs["text_embed"], f32)
    W_ada, b_ada = np.asarray(inputs["W_ada"], f32), np.asarray(inputs["b_ada"], f32)
    W_qkv, b_qkv = np.asarray(inputs["W_qkv"], f32), np.asarray(inputs["b_qkv"], f32)
    W_proj, b_proj = np.asarray(inputs["W_proj"], f32), np.asarray(inputs["b_proj"], f32)
    W_ctx, b_ctx = np.asarray(inputs["W_ctx"], f32), np.asarray(inputs["b_ctx"], f32)
    W_q, W_k, W_v = (np.asarray(inputs[k], f32) for k in ("W_q", "W_k", "W_v"))
    W_out, b_out = np.asarray(inputs["W_out"], f32), np.asarray(inputs["b_out"], f32)
    W_fc1, b_fc1 = np.asarray(inputs["W_fc1"], f32), np.asarray(inputs["b_fc1"], f32)
    W_fc2, b_fc2 = np.asarray(inputs["W_fc2"], f32), np.asarray(inputs["b_fc2"], f32)

    maps = []
    for core in range(8):
        b, hf = core // 2, core % 2
        sl = slice(DL * hf, DL * (hf + 1))
        half = (lambda a: a) if hf == 0 else (lambda a: np.zeros_like(a))
        qs = slice(DL * hf, DL * (hf + 1))
        ks_ = slice(D + DL * hf, D + DL * (hf + 1))
        vs = slice(2 * D + DL * hf, 2 * D + DL * (hf + 1))
        wq_p = W_qkv[:, qs][:, PERM]
        wk_p = W_qkv[:, ks_][:, PERM]
        w1 = W_fc1[:, FFL * hf:FFL * (hf + 1)]
        w1h = w1.astype(f8)
        w1l = ((w1 - w1h.astype(f32)) * 64).astype(f8)
        m = {
            "xT": np.ascontiguousarray(x[b].T).astype(bf16),
            "cT": np.ascontiguousarray(c[b, sl].reshape(4, 128).T),
            "teT": np.ascontiguousarray(np.pad(te[b].T, ((0, 0), (0, TLP - TL)))).astype(bf16),
            "w_ada": pretile(W_ada[sl, :].astype(bf16), 512),
            "b_adaT": np.ascontiguousarray(b_ada.reshape(6, 8, 128).transpose(2, 0, 1)),
            "w_qkv": pretile(np.concatenate(
                [wq_p, wk_p, W_qkv[:, vs]], axis=1).astype(f8), 256),
            "b_qkT": np.ascontiguousarray(
                np.concatenate([b_qkv[qs][PERM], b_qkv[ks_][PERM]]).reshape(8, 128).T),
            "b_v": b_qkv[vs][None, :],
            "w_proj": pretile(W_proj[sl, :].astype(bf16), 512),
            "b_projT": np.ascontiguousarray(half(b_proj).reshape(8, 128).T),
            "w_ctx": pretile(W_ctx.astype(bf16), 256),
            "b_ctxT": np.ascontiguousarray(b_ctx.reshape(8, 128).T),
            "w_q": pretile(W_q[:, sl][:, PERM].astype(f8), 256),
            "w_k": pretile(W_k[:, sl][:, PERM].astype(f8), 256),
            "w_v": pretile(W_v[:, sl].astype(f8), 256),
            "w_out": pretile(W_out[sl, :].astype(bf16), 512),
            "b_outT": np.ascontiguousarray(half(b_out).reshape(8, 128).T),
            "w_fc1h": pretile(w1h, 256),
            "w_fc1l": pretile(w1l, 256),
            "b_fc1T": np.ascontiguousarray(
                b_fc1[FFL * hf:FFL * (hf + 1)].reshape(16, 128).T),
            "w_fc2": pretile(W_fc2[FFL * hf:FFL * (hf + 1), :].astype(bf16), 256),
            "b_fc2T": np.ascontiguousarray(half(b_fc2).reshape(8, 128).T),
            "ones_r": np.ones((128, 128), f32),
            "ones_b": np.ones((128, 8), bf16),
            "maskT": np.full((128, 1), 1.0 - hf, f32),
        }
        maps.append(m)
    return maps


_NC_CACHE = None


def kernel(**inputs):
    global _NC_CACHE
    if _NC_CACHE is None:
        _NC_CACHE = build()
    nc = _NC_CACHE
    in_maps = shard_inputs(inputs)
    res = run_bass_kernel_spmd(nc, in_maps, core_ids=list(range(8)))
    out = np.empty((B, N, D), np.float32)
    for b in range(B):
        p0 = res.results[2 * b]["out_xT"]
        p1 = res.results[2 * b + 1]["out_xT"]
        out[b] = (p0.astype(np.float32) + p1.astype(np.float32)).T
    return out
